# revision 18
# baseline (speedup 1.0000x reference)
"""GPTQ-style 4-bit quantized linear (x @ dequant(qweight) + bias) on 8 TRN2 cores.

Column-parallel: N=11008 sharded across 8 cores (1376 each, padded to
1408 = 4 planes x 352). Host prep is bit-layout repacking only:
 - nibbles repacked along N (8 per int32 word) so a word unpacks to
   same-k different-n values, then k-tiled to [128, G*NW] (partition-major)
 - x transposed/tiled to [128, G*16]
 - scales permuted to the plane-major column order the unpack produces

Device kernel per core:
 1. Unpack: fused (and, or) tensor_scalar ops -> fp16 planes in place:
    value = 1024 + E*q (E in {1,16}), exponent 0x6400. The >>8 shift is a
    1-byte-offset DMA copy. G/CH chunks x (1 shift-DMA + 4 plane ops).
 2. Mains (fp16): lhsT = xT_g [128,32] (tokens padded), rhs = plane
    [128,352], 4 planes col-tiled -> per-group partials; PSUM not
    accumulated across groups (GPTQ group scales differ).
 3. Evac: per-group tensor_scalar/ACT-activation subtracts the 1024-offset
    (per-partition bias = -1024*xsum) and writes fp16; a single merged DMA
    per group remaps [(32j+t), w] -> [g, (j,t,w)].
 4. Scale matmul (fp16) contracts groups: lhsT = (s/E)-window [32,32],
    rhs free order (w, t) so the diagonal = contiguous 16-element runs.
 5. Correction matmul C^T[n,t] = sum_g SZ[g,n]*xsum[t,g] - bias[n].
 6. scP -> SBUF fp16 -> DRAM scratch -> flat diag DMA -> subtract -> out.

Math: out[t,n] = sum_g s'[g,n]*S'_g[t,n] - (sum_g SZ[g,n]*xsum[t,g] - bias)
  S'_g = sum_{k in g} x_k*E*q (offset removed at evac), s' = fp16(s/E),
  SZ = s'*zenc + (s - 1024*s'), zenc = fp16-encoded zeros plane.
"""

import numpy as np
from contextlib import ExitStack

import concourse.bass as bass
import concourse.tile as tile
from concourse import mybir, bacc
from concourse.alu_op_type import AluOpType
from concourse.bass_utils import run_bass_kernel_spmd
from concourse.masks import make_identity

MASK_LO = 0x000F000F
MASK_HI = 0x00F000F0
EXP16 = 0x64006400
N_CORES = 8
GROUPSIZE = 128


class Cfg:
    def __init__(self, K=4096, N_shard=1376, T=16, chunk=8):
        self.K = K
        self.G = K // GROUPSIZE
        self.T = T
        self.N_shard = N_shard
        per_plane = -(-N_shard // 8) * 2
        self.PW = -(-per_plane // 32) * 32
        self.NPAD = 4 * self.PW
        self.NW = self.NPAD // 8
        self.R = self.PW // 32
        self.CH = min(chunk, self.G)      # groups per unpack chunk
        assert self.G % self.CH == 0 and N_shard % 8 == 0


FULL = Cfg()

# ---------------------------------------------------------------- host prep


def _unpack_rows(packed, rows):
    w = packed.view(np.uint32)
    out = np.empty((rows, packed.shape[1]), dtype=np.uint8)
    for b in range(8):
        out[b::8] = ((w >> np.uint32(4 * b)) & np.uint32(0xF)).astype(np.uint8)
    return out


def _unpack_cols(packed):
    w = packed.view(np.uint32)
    out = np.empty((w.shape[0], w.shape[1] * 8), dtype=np.uint8)
    for b in range(8):
        out[:, b::8] = ((w >> np.uint32(4 * b)) & np.uint32(0xF)).astype(np.uint8)
    return out


def _pack_cols(nib):
    w = np.zeros((nib.shape[0], nib.shape[1] // 8), dtype=np.uint32)
    for b in range(8):
        w |= nib[:, b::8].astype(np.uint32) << np.uint32(4 * b)
    return w.view(np.int32)


def _perm(cfg):
    p = np.empty(cfg.NPAD, dtype=np.int64)
    m = np.arange(cfg.PW // 2)
    for j in range(4):
        for h in range(2):
            p[j * cfg.PW + 2 * m + h] = 8 * m + j + 4 * h
    return p


def _escale(cfg):
    e = np.ones(cfg.NPAD, dtype=np.float32)
    e[cfg.PW:2 * cfg.PW] = 16.0
    e[3 * cfg.PW:] = 16.0
    return e


def host_prep(cfg, x, qweight, qzeros, scales, bias):
    nib = _unpack_rows(np.asarray(qweight), cfg.K)
    znib = _unpack_cols(np.asarray(qzeros))
    perm, e = _perm(cfg), _escale(cfg)
    x = np.asarray(x, dtype=np.float32)
    # xt[p, g*T+t] = x[t, g*128+p]
    xt = np.ascontiguousarray(
        x.reshape(cfg.T, cfg.G, 128).transpose(2, 1, 0).reshape(128, cfg.G * cfg.T)
    )
    in_maps = []
    for c in range(N_CORES):
        sl = slice(c * cfg.N_shard, (c + 1) * cfg.N_shard)
        nib_s = np.zeros((cfg.K, cfg.NPAD), dtype=np.uint8)
        nib_s[:, : cfg.N_shard] = nib[:, sl]
        znib_s = np.zeros((cfg.G, cfg.NPAD), dtype=np.uint8)
        znib_s[:, : cfg.N_shard] = znib[:, sl]
        s_s = np.zeros((cfg.G, cfg.NPAD), dtype=np.float32)
        s_s[:, : cfg.N_shard] = scales[:, sl]
        b_s = np.zeros(cfg.NPAD, dtype=np.float32)
        b_s[: cfg.N_shard] = bias[sl]
        qw2 = _pack_cols(nib_s)  # [K, NW]
        # partition-major tiling: qwt[p, g*NW+m] = qw2[g*128+p, m]
        # chunk-major: qwt row-block for chunk c is fully contiguous in DRAM
        qwt = np.ascontiguousarray(
            qw2.reshape(cfg.G // cfg.CH, cfg.CH, 128, cfg.NW)
            .transpose(0, 2, 1, 3)
            .reshape(cfg.G // cfg.CH, 128, cfg.CH * cfg.NW)
        ).reshape(128 * (cfg.G // cfg.CH), cfg.CH * cfg.NW)
        s_p = s_s[:, perm]
        spv = (s_p / e[None, :]).astype(np.float16)
        in_maps.append(
            {
                "qw": qwt,
                "qz": _pack_cols(znib_s),
                "sp": spv,
                "s2": (s_p - 1024.0 * spv.astype(np.float64)).astype(np.float32),
                "biasp": (-b_s[perm]).astype(np.float32),
                "xt": xt,
            }
        )
    return in_maps


def host_gather(cfg, results):
    perm = _perm(cfg)
    valid = perm < cfg.N_shard
    out = np.empty((cfg.T, cfg.N_shard * N_CORES), dtype=np.float32)
    for c in range(N_CORES):
        oT = results[c]["outT"]
        shard = np.empty((cfg.T, cfg.N_shard), dtype=np.float32)
        shard[:, perm[valid]] = oT[valid].T
        out[:, c * cfg.N_shard:(c + 1) * cfg.N_shard] = shard
    return out


# ---------------------------------------------------------------- device kernel


def build_kernel(nc, cfg):
    f32, f16, i32 = mybir.dt.float32, mybir.dt.float16, mybir.dt.int32
    u8 = mybir.dt.uint8
    G, T, PW, NW, R, CH = cfg.G, cfg.T, cfg.PW, cfg.NW, cfg.R, cfg.CH
    NPAD = cfg.NPAD
    TPW = T * PW            # one (g, j) plane in rhsbig

    qw_d = nc.declare_dram_parameter("qw", [128 * (G // CH), CH * NW], i32, isOutput=False)
    qz_d = nc.declare_dram_parameter("qz", [G, NW], i32, isOutput=False)
    sp_d = nc.declare_dram_parameter("sp", [G, NPAD], f16, isOutput=False)
    s2_d = nc.declare_dram_parameter("s2", [G, NPAD], f32, isOutput=False)
    bias_d = nc.declare_dram_parameter("biasp", [NPAD], f32, isOutput=False)
    xt_d = nc.declare_dram_parameter("xt", [128, G * T], f32, isOutput=False)
    out_d = nc.declare_dram_parameter("outT", [NPAD, T], f32, isOutput=True)
    scr_d = nc.dram_tensor("scratch", [R, 128, 512], f16).ap()
    sco_d = nc.dram_tensor("scopy_dram", [128, G * PW], f16).ap()

    dmae = [nc.sync, nc.scalar]  # the two HWDGE rings

    def dq(i):
        return dmae[i % len(dmae)]

    with tile.TileContext(nc) as tc, ExitStack() as ctx:
        singles = ctx.enter_context(tc.tile_pool(name="singles", bufs=1))
        qwp = ctx.enter_context(tc.tile_pool(name="qwp", bufs=2))
        encp = ctx.enter_context(tc.tile_pool(name="encp", bufs=2))
        smallp = ctx.enter_context(tc.tile_pool(name="smallp", bufs=2))
        ps_main = ctx.enter_context(tc.tile_pool(name="ps_main", bufs=1, space="PSUM"))
        ps_sc = ctx.enter_context(tc.tile_pool(name="ps_sc", bufs=2, space="PSUM"))
        ps_c = ctx.enter_context(tc.tile_pool(name="ps_c", bufs=2, space="PSUM"))

        # ---------- phase 0: x prep ----------
        xf = singles.tile([128, G * T], f32)
        nc.sync.dma_start(out=xf[:], in_=xt_d[:])
        # tokens padded to 32 per group so matmuls write full PSUM strips
        xT = singles.tile([128, G * 32], f16)
        nc.vector.memset(xT[:], 0.0)
        nc.vector.tensor_copy(
            xT[:].rearrange("p (g t) -> p g t", g=G)[:, :, 0:T], xf[:]
        )
        ones16 = singles.tile([128, 1], f16)
        nc.vector.memset(ones16[:], 1.0)

        xsumP = ps_sc.tile([T, G], f32, tag="sc")
        for g in range(G):
            nc.tensor.matmul(
                xsumP[:, g:g + 1], xT[:, g * 32:g * 32 + T], ones16[:],
                start=True, stop=True,
            )
        xsum_s = singles.tile([T, G], f32)
        nc.scalar.copy(xsum_s[:], xsumP[:])
        ident = singles.tile([T, T], f32)
        make_identity(nc, ident[:])
        xsT_P = ps_sc.tile([G, T], f32, tag="sc")
        nc.tensor.transpose(xsT_P[:], xsum_s[:], ident[:])
        xsum_aug = singles.tile([G + 1, T], f32)
        nc.vector.memset(xsum_aug[:], 1.0)
        nc.scalar.copy(xsum_aug[:G, :], xsT_P[:])
        # offv[32j+t, g] = -1024 * xsum[t, g]
        offv = singles.tile([128, G], f32)
        nc.vector.memset(offv[:], 0.0)
        for j in range(4):
            nc.scalar.mul(offv[32 * j:32 * j + T, :], xsumP[:], -1024.0)

        # ---------- phase 1: scales / zeros prep ----------
        sp16 = singles.tile([G, NPAD], f16)
        nc.sync.dma_start(out=sp16[:], in_=sp_d[:])
        sp_s = singles.tile([G, NPAD], f32)
        nc.vector.tensor_copy(sp_s[:], sp16[:])
        s_s = singles.tile([G, NPAD], f32)
        nc.sync.dma_start(out=s_s[:], in_=s2_d[:])

        qz_t = singles.tile([G, NW], i32)
        nc.sync.dma_start(out=qz_t[:], in_=qz_d[:])
        zs = singles.tile([G, NW], i32)
        nc.vector.memset(zs[:, NW - 1:NW], 0)
        nc.sync.dma_start(
            out=zs[:].bitcast(u8)[:, 0:4 * NW - 1],
            in_=qz_t[:].bitcast(u8)[:, 1:4 * NW],
        )
        zenc = singles.tile([G, NPAD], f16)
        zi = zenc[:].bitcast(i32)
        for j, (src, mask) in enumerate(
            [(qz_t, MASK_LO), (qz_t, MASK_HI), (zs, MASK_LO), (zs, MASK_HI)]
        ):
            nc.vector.tensor_scalar(
                out=zi[:, j * NW:(j + 1) * NW], in0=src[:],
                scalar1=mask, scalar2=EXP16,
                op0=AluOpType.bitwise_and, op1=AluOpType.bitwise_or,
            )
        zf = singles.tile([G, NPAD], f32)
        nc.vector.tensor_copy(zf[:], zenc[:])
        szb = singles.tile([G + 1, NPAD], f32)
        nc.vector.tensor_tensor(szb[:G, :], zf[:], sp_s[:], AluOpType.mult)
        nc.vector.tensor_tensor(szb[:G, :], szb[:G, :], s_s[:], AluOpType.add)
        nc.sync.dma_start(out=szb[G:G + 1, :], in_=bias_d[None, :])

        # ---------- phase 2: unpack + mains + evac ----------
        rhsbig = singles.tile([G, 4 * TPW], f16)
        scopy = singles.tile([128, G * PW], f16)
        nd = 0  # DMA ring round-robin counter
        for c0 in range(0, G, CH):
            wt = qwp.tile([128, CH * NW], i32, tag="wt")
            nc.sync.dma_start(
                out=wt[:], in_=qw_d[(c0 // CH) * 128:(c0 // CH + 1) * 128, :]
            )
            ws = qwp.tile([128, CH * NW], i32, tag="ws")
            nc.vector.memset(ws[:, CH * NW - 1:CH * NW], 0)
            nc.scalar.dma_start(
                out=ws[:].bitcast(u8)[:, 0:4 * CH * NW - 1],
                in_=wt[:].bitcast(u8)[:, 1:4 * CH * NW],
            )
            enc = encp.tile([128, CH * NPAD], f16, tag="enc")
            ei = enc[:].bitcast(i32)
            for j, (src, mask) in enumerate(
                [(wt, MASK_LO), (wt, MASK_HI), (ws, MASK_LO), (ws, MASK_HI)]
            ):
                nc.vector.tensor_scalar(
                    out=ei[:, j * CH * NW:(j + 1) * CH * NW],
                    in0=src[:],
                    scalar1=mask, scalar2=EXP16,
                    op0=AluOpType.bitwise_and, op1=AluOpType.bitwise_or,
                )
            for gg in range(CH):
                g = c0 + gg
                mainP = ps_main.tile(
                    [128, 512], f32, tag=f"m{g % 4}", name=f"mainP{g % 4}"
                )
                for j in range(4):
                    nc.tensor.matmul(
                        mainP[32 * j:32 * (j + 1), 0:PW],
                        xT[:, g * 32:(g + 1) * 32],
                        enc[:, (j * CH + gg) * PW:(j * CH + gg + 1) * PW],
                        start=True, stop=True, tile_position=(0, 32 * j),
                    )
                # evac: subtract offset, fp16 (alternate DVE / ACT)
                if g % 2 == 0:
                    nc.vector.tensor_scalar(
                        out=scopy[:, g * PW:(g + 1) * PW], in0=mainP[:, 0:PW],
                        scalar1=offv[:, g:g + 1], scalar2=None,
                        op0=AluOpType.add,
                    )
                else:
                    nc.scalar.activation(
                        scopy[:, g * PW:(g + 1) * PW], mainP[:, 0:PW],
                        mybir.ActivationFunctionType.Identity,
                        bias=offv[:, g:g + 1], scale=1.0,
                    )

            # chunk's evac window -> DRAM mirror (contiguous per partition)
            dq(nd).dma_start(
                out=sco_d[:, c0 * PW:(c0 + CH) * PW],
                in_=scopy[:, c0 * PW:(c0 + CH) * PW],
            )
            nd += 1

        # remap gather: DRAM -> [g, (j, t, w)] tiles, one DMA per plane j
        for j in range(4):
            dq(nd).dma_start(
                out=rhsbig[:, j * TPW:(j + 1) * TPW].rearrange(
                    "g (t w) -> g t w", t=T
                ),
                in_=bass.AP(
                    tensor=sco_d.tensor,
                    offset=sco_d.offset + (32 * j) * (G * PW),
                    ap=[[PW, G], [G * PW, T], [1, PW]],
                ),
            )
            nd += 1

        # ---------- phase 3: scale matmul + correction + diag out ----------
        for r in range(R):
            scP = ps_sc.tile([128, 512], f32, tag="sc", name=f"scP{r}")
            cP = ps_c.tile([128, T], f32, tag="c", name=f"cP{r}")
            for j in range(4):
                rhs_ap = rhsbig[:, :].rearrange(
                    "g (j t w) -> g j w t", j=4, t=T
                )[:, j, 32 * r:32 * r + 32, :]
                nc.tensor.matmul(
                    scP[32 * j:32 * (j + 1), :],
                    sp16[:, j * PW + 32 * r: j * PW + 32 * r + 32],
                    rhs_ap,
                    start=True, stop=True, tile_position=(0, 32 * j),
                )
                nc.tensor.matmul(
                    cP[32 * j:32 * (j + 1), :],
                    szb[:, j * PW + 32 * r: j * PW + 32 * r + 32],
                    xsum_aug[:],
                    start=True, stop=True, tile_position=(0, 32 * j),
                )
            scS = smallp.tile([128, 512], f16, tag="scS")
            if r % 2 == 0:
                nc.vector.tensor_copy(scS[:], scP[:])
            else:
                nc.scalar.copy(scS[:], scP[:])
            dq(nd).dma_start(out=scr_d[r], in_=scS[:])
            nd += 1
            diagbuf = smallp.tile([128, T], f16, tag="diagbuf")
            diag_src = bass.AP(
                tensor=scr_d.tensor,
                offset=scr_d.offset + r * 128 * 512,
                ap=[[32 * 512, 4], [512 + 16, 32], [1, T]],
            )
            nc.gpsimd.dma_start(out=diagbuf[:], in_=diag_src)
            oT = smallp.tile([128, T], f32, tag="oT")
            nc.vector.scalar_tensor_tensor(
                out=oT[:], in0=diagbuf[:], scalar=0.0, in1=cP[:],
                op0=AluOpType.bypass, op1=AluOpType.subtract,
            )
            nc.gpsimd.dma_start(
                out=out_d[:].rearrange("(j w) t -> j w t", j=4)[
                    :, 32 * r:32 * r + 32, :
                ],
                in_=oT[:],
            )
    return nc


# ---------------------------------------------------------------- entry

_CACHE = {}


def _get_nc(cfg):
    key = (cfg.K, cfg.NPAD, cfg.T)
    if key not in _CACHE:
        nc = bacc.Bacc(num_devices=N_CORES)
        build_kernel(nc, cfg)
        nc.compile()
        _CACHE[key] = nc
    return _CACHE[key]


def kernel(x, qweight, qzeros, scales, bias):
    cfg = FULL
    in_maps = host_prep(cfg, x, qweight, qzeros, scales, bias)
    nc = _get_nc(cfg)
    res = run_bass_kernel_spmd(nc, in_maps, core_ids=list(range(N_CORES)))
    return host_gather(cfg, res.results)


# revision 21
# speedup vs baseline: 13.2099x; 13.2099x over previous
"""GPTQ-style 4-bit quantized linear (x @ dequant(qweight) + bias) on 8 TRN2 cores.

Column-parallel: N=11008 sharded across 8 cores (1376 each, padded to
1408 = 4 planes x 352). Host prep is bit-layout repacking only:
 - nibbles repacked along N (8 per int32 word) so a word unpacks to
   same-k different-n values, then k-tiled to [128, G*NW] (partition-major)
 - x transposed/tiled to [128, G*16]
 - scales permuted to the plane-major column order the unpack produces

Device kernel per core:
 1. Unpack: fused (and, or) tensor_scalar ops -> fp16 planes in place:
    value = 1024 + E*q (E in {1,16}), exponent 0x6400. The >>8 shift is a
    1-byte-offset DMA copy. G/CH chunks x (1 shift-DMA + 4 plane ops).
 2. Mains (fp16): lhsT = xT_g [128,32] (tokens padded), rhs = plane
    [128,352], 4 planes col-tiled -> per-group partials; PSUM not
    accumulated across groups (GPTQ group scales differ).
 3. Evac: per-group tensor_scalar/ACT-activation subtracts the 1024-offset
    (per-partition bias = -1024*xsum) and writes fp16; a single merged DMA
    per group remaps [(32j+t), w] -> [g, (j,t,w)].
 4. Scale matmul (fp16) contracts groups: lhsT = (s/E)-window [32,32],
    rhs free order (w, t) so the diagonal = contiguous 16-element runs.
 5. Correction matmul C^T[n,t] = sum_g SZ[g,n]*xsum[t,g] - bias[n].
 6. scP -> SBUF fp16 -> DRAM scratch -> flat diag DMA -> subtract -> out.

Math: out[t,n] = sum_g s'[g,n]*S'_g[t,n] - (sum_g SZ[g,n]*xsum[t,g] - bias)
  S'_g = sum_{k in g} x_k*E*q (offset removed at evac), s' = fp16(s/E),
  SZ = s'*zenc + (s - 1024*s'), zenc = fp16-encoded zeros plane.
"""

import numpy as np
from contextlib import ExitStack

import concourse.bass as bass
import concourse.tile as tile
from concourse import mybir, bacc
from concourse.alu_op_type import AluOpType
from concourse.bass_utils import run_bass_kernel_spmd
from concourse.masks import make_identity

MASK_LO = 0x000F000F
MASK_HI = 0x00F000F0
EXP16 = 0x64006400
N_CORES = 8
GROUPSIZE = 128


class Cfg:
    def __init__(self, K=4096, N_shard=1376, T=16, chunk=8):
        self.K = K
        self.G = K // GROUPSIZE
        self.T = T
        self.N_shard = N_shard
        per_plane = -(-N_shard // 8) * 2
        self.PW = -(-per_plane // 32) * 32
        self.NPAD = 4 * self.PW
        self.NW = self.NPAD // 8
        self.R = self.PW // 32
        self.CH = min(chunk, self.G)      # groups per unpack chunk
        assert self.G % self.CH == 0 and N_shard % 8 == 0


FULL = Cfg()

# ---------------------------------------------------------------- host prep


def _unpack_rows(packed, rows):
    w = packed.view(np.uint32)
    out = np.empty((rows, packed.shape[1]), dtype=np.uint8)
    for b in range(8):
        out[b::8] = ((w >> np.uint32(4 * b)) & np.uint32(0xF)).astype(np.uint8)
    return out


def _unpack_cols(packed):
    w = packed.view(np.uint32)
    out = np.empty((w.shape[0], w.shape[1] * 8), dtype=np.uint8)
    for b in range(8):
        out[:, b::8] = ((w >> np.uint32(4 * b)) & np.uint32(0xF)).astype(np.uint8)
    return out


def _pack_cols(nib):
    w = np.zeros((nib.shape[0], nib.shape[1] // 8), dtype=np.uint32)
    for b in range(8):
        w |= nib[:, b::8].astype(np.uint32) << np.uint32(4 * b)
    return w.view(np.int32)


def _perm(cfg):
    p = np.empty(cfg.NPAD, dtype=np.int64)
    m = np.arange(cfg.PW // 2)
    for j in range(4):
        for h in range(2):
            p[j * cfg.PW + 2 * m + h] = 8 * m + j + 4 * h
    return p


def _escale(cfg):
    e = np.ones(cfg.NPAD, dtype=np.float32)
    e[cfg.PW:2 * cfg.PW] = 16.0
    e[3 * cfg.PW:] = 16.0
    return e


def host_prep(cfg, x, qweight, qzeros, scales, bias):
    nib = _unpack_rows(np.asarray(qweight), cfg.K)
    znib = _unpack_cols(np.asarray(qzeros))
    perm, e = _perm(cfg), _escale(cfg)
    x = np.asarray(x, dtype=np.float32)
    # xt[p, g*T+t] = x[t, g*128+p]
    xt = np.ascontiguousarray(
        x.reshape(cfg.T, cfg.G, 128).transpose(2, 1, 0).reshape(128, cfg.G * cfg.T)
    )
    in_maps = []
    for c in range(N_CORES):
        sl = slice(c * cfg.N_shard, (c + 1) * cfg.N_shard)
        nib_s = np.zeros((cfg.K, cfg.NPAD), dtype=np.uint8)
        nib_s[:, : cfg.N_shard] = nib[:, sl]
        znib_s = np.zeros((cfg.G, cfg.NPAD), dtype=np.uint8)
        znib_s[:, : cfg.N_shard] = znib[:, sl]
        s_s = np.zeros((cfg.G, cfg.NPAD), dtype=np.float32)
        s_s[:, : cfg.N_shard] = scales[:, sl]
        b_s = np.zeros(cfg.NPAD, dtype=np.float32)
        b_s[: cfg.N_shard] = bias[sl]
        qw2 = _pack_cols(nib_s)  # [K, NW]
        # partition-major tiling: qwt[p, g*NW+m] = qw2[g*128+p, m]
        # chunk-major: qwt row-block for chunk c is fully contiguous in DRAM
        qwt = np.ascontiguousarray(
            qw2.reshape(cfg.G // cfg.CH, cfg.CH, 128, cfg.NW)
            .transpose(0, 2, 1, 3)
            .reshape(cfg.G // cfg.CH, 128, cfg.CH * cfg.NW)
        ).reshape(128 * (cfg.G // cfg.CH), cfg.CH * cfg.NW)
        s_p = s_s[:, perm]
        spv = (s_p / e[None, :]).astype(np.float16)
        in_maps.append(
            {
                "qw": qwt,
                "qz": _pack_cols(znib_s),
                "sp": spv,
                "s2": (s_p - 1024.0 * spv.astype(np.float64)).astype(np.float32),
                "biasp": (-b_s[perm]).astype(np.float32),
                "xt": xt,
            }
        )
    return in_maps


def host_gather(cfg, results):
    perm = _perm(cfg)
    valid = perm < cfg.N_shard
    out = np.empty((cfg.T, cfg.N_shard * N_CORES), dtype=np.float32)
    for c in range(N_CORES):
        oT = results[c]["outT"]
        shard = np.empty((cfg.T, cfg.N_shard), dtype=np.float32)
        shard[:, perm[valid]] = oT[valid].T
        out[:, c * cfg.N_shard:(c + 1) * cfg.N_shard] = shard
    return out


# ---------------------------------------------------------------- device kernel


def build_kernel(nc, cfg, reps=1):
    f32, f16, i32 = mybir.dt.float32, mybir.dt.float16, mybir.dt.int32
    u8 = mybir.dt.uint8
    G, T, PW, NW, R, CH = cfg.G, cfg.T, cfg.PW, cfg.NW, cfg.R, cfg.CH
    NPAD = cfg.NPAD
    TPW = T * PW            # one (g, j) plane in rhsbig

    qw_d = nc.declare_dram_parameter("qw", [128 * (G // CH), CH * NW], i32, isOutput=False)
    qz_d = nc.declare_dram_parameter("qz", [G, NW], i32, isOutput=False)
    sp_d = nc.declare_dram_parameter("sp", [G, NPAD], f16, isOutput=False)
    s2_d = nc.declare_dram_parameter("s2", [G, NPAD], f32, isOutput=False)
    bias_d = nc.declare_dram_parameter("biasp", [NPAD], f32, isOutput=False)
    xt_d = nc.declare_dram_parameter("xt", [128, G * T], f32, isOutput=False)
    out_d = nc.declare_dram_parameter("outT", [NPAD, T], f32, isOutput=True)
    scr_d = nc.dram_tensor("scratch", [R, 128, 512], f16).ap()
    sco_d = nc.dram_tensor("scopy_dram", [128, G * PW], f16).ap()

    dmae = [nc.sync, nc.scalar]  # the two HWDGE rings

    def dq(i):
        return dmae[i % len(dmae)]

    with tile.TileContext(nc) as tc:
      for rep in range(reps):
       with ExitStack() as ctx:
        singles = ctx.enter_context(tc.tile_pool(name=f"singles{rep}", bufs=1))
        qwp = ctx.enter_context(tc.tile_pool(name=f"qwp{rep}", bufs=2))
        encp = ctx.enter_context(tc.tile_pool(name=f"encp{rep}", bufs=2))
        smallp = ctx.enter_context(tc.tile_pool(name=f"smallp{rep}", bufs=2))
        ps_main = ctx.enter_context(tc.tile_pool(name=f"ps_main{rep}", bufs=1, space="PSUM"))
        ps_sc = ctx.enter_context(tc.tile_pool(name=f"ps_sc{rep}", bufs=2, space="PSUM"))
        ps_c = ctx.enter_context(tc.tile_pool(name=f"ps_c{rep}", bufs=2, space="PSUM"))

        # ---------- phase 0: x prep ----------
        xf = singles.tile([128, G * T], f32)
        nc.sync.dma_start(out=xf[:], in_=xt_d[:])
        # tokens padded to 32 per group so matmuls write full PSUM strips
        xT = singles.tile([128, G * 32], f16)
        nc.vector.memset(xT[:], 0.0)
        nc.vector.tensor_copy(
            xT[:].rearrange("p (g t) -> p g t", g=G)[:, :, 0:T], xf[:]
        )
        ones16 = singles.tile([128, 1], f16)
        nc.vector.memset(ones16[:], 1.0)

        xsumP = ps_sc.tile([T, G], f32, tag="sc")
        for g in range(G):
            nc.tensor.matmul(
                xsumP[:, g:g + 1], xT[:, g * 32:g * 32 + T], ones16[:],
                start=True, stop=True,
            )
        xsum_s = singles.tile([T, G], f32)
        nc.scalar.copy(xsum_s[:], xsumP[:])
        ident = singles.tile([T, T], f32)
        make_identity(nc, ident[:])
        xsT_P = ps_sc.tile([G, T], f32, tag="sc")
        nc.tensor.transpose(xsT_P[:], xsum_s[:], ident[:])
        xsum_aug = singles.tile([G + 1, T], f32)
        nc.vector.memset(xsum_aug[:], 1.0)
        nc.scalar.copy(xsum_aug[:G, :], xsT_P[:])
        # offv[32j+t, g] = -1024 * xsum[t, g]
        offv = singles.tile([128, G], f32)
        nc.vector.memset(offv[:], 0.0)
        for j in range(4):
            nc.scalar.mul(offv[32 * j:32 * j + T, :], xsumP[:], -1024.0)

        # ---------- phase 1: scales / zeros prep ----------
        sp16 = singles.tile([G, NPAD], f16)
        nc.sync.dma_start(out=sp16[:], in_=sp_d[:])
        sp_s = singles.tile([G, NPAD], f32)
        nc.vector.tensor_copy(sp_s[:], sp16[:])
        s_s = singles.tile([G, NPAD], f32)
        nc.sync.dma_start(out=s_s[:], in_=s2_d[:])

        qz_t = singles.tile([G, NW], i32)
        nc.sync.dma_start(out=qz_t[:], in_=qz_d[:])
        zs = singles.tile([G, NW], i32)
        nc.vector.memset(zs[:, NW - 1:NW], 0)
        nc.sync.dma_start(
            out=zs[:].bitcast(u8)[:, 0:4 * NW - 1],
            in_=qz_t[:].bitcast(u8)[:, 1:4 * NW],
        )
        zenc = singles.tile([G, NPAD], f16)
        zi = zenc[:].bitcast(i32)
        for j, (src, mask) in enumerate(
            [(qz_t, MASK_LO), (qz_t, MASK_HI), (zs, MASK_LO), (zs, MASK_HI)]
        ):
            nc.vector.tensor_scalar(
                out=zi[:, j * NW:(j + 1) * NW], in0=src[:],
                scalar1=mask, scalar2=EXP16,
                op0=AluOpType.bitwise_and, op1=AluOpType.bitwise_or,
            )
        zf = singles.tile([G, NPAD], f32)
        nc.vector.tensor_copy(zf[:], zenc[:])
        szb = singles.tile([G + 1, NPAD], f32)
        nc.vector.tensor_tensor(szb[:G, :], zf[:], sp_s[:], AluOpType.mult)
        nc.vector.tensor_tensor(szb[:G, :], szb[:G, :], s_s[:], AluOpType.add)
        nc.sync.dma_start(out=szb[G:G + 1, :], in_=bias_d[None, :])

        # ---------- phase 2: unpack + mains + evac ----------
        rhsbig = singles.tile([G, 4 * TPW], f16)
        scopy = singles.tile([128, G * PW], f16)
        nd = 0  # DMA ring round-robin counter
        for c0 in range(0, G, CH):
            wt = qwp.tile([128, CH * NW], i32, tag="wt")
            nc.sync.dma_start(
                out=wt[:], in_=qw_d[(c0 // CH) * 128:(c0 // CH + 1) * 128, :]
            )
            ws = qwp.tile([128, CH * NW], i32, tag="ws")
            nc.vector.memset(ws[:, CH * NW - 1:CH * NW], 0)
            nc.scalar.dma_start(
                out=ws[:].bitcast(u8)[:, 0:4 * CH * NW - 1],
                in_=wt[:].bitcast(u8)[:, 1:4 * CH * NW],
            )
            enc = encp.tile([128, CH * NPAD], f16, tag="enc")
            ei = enc[:].bitcast(i32)
            for j, (src, mask) in enumerate(
                [(wt, MASK_LO), (wt, MASK_HI), (ws, MASK_LO), (ws, MASK_HI)]
            ):
                nc.vector.tensor_scalar(
                    out=ei[:, j * CH * NW:(j + 1) * CH * NW],
                    in0=src[:],
                    scalar1=mask, scalar2=EXP16,
                    op0=AluOpType.bitwise_and, op1=AluOpType.bitwise_or,
                )
            for gg in range(CH):
                g = c0 + gg
                mainP = ps_main.tile(
                    [128, 512], f32, tag=f"m{g % 4}", name=f"mainP{rep}_{g % 4}"
                )
                for j in range(4):
                    nc.tensor.matmul(
                        mainP[32 * j:32 * (j + 1), 0:PW],
                        xT[:, g * 32:(g + 1) * 32],
                        enc[:, (j * CH + gg) * PW:(j * CH + gg + 1) * PW],
                        start=True, stop=True, tile_position=(0, 32 * j),
                    )
                # evac: subtract offset, fp16 (alternate DVE / ACT)
                if g % 2 == 0:
                    nc.vector.tensor_scalar(
                        out=scopy[:, g * PW:(g + 1) * PW], in0=mainP[:, 0:PW],
                        scalar1=offv[:, g:g + 1], scalar2=None,
                        op0=AluOpType.add,
                    )
                else:
                    nc.scalar.activation(
                        scopy[:, g * PW:(g + 1) * PW], mainP[:, 0:PW],
                        mybir.ActivationFunctionType.Identity,
                        bias=offv[:, g:g + 1], scale=1.0,
                    )

            # chunk's evac window -> DRAM mirror (contiguous per partition)
            dq(nd).dma_start(
                out=sco_d[:, c0 * PW:(c0 + CH) * PW],
                in_=scopy[:, c0 * PW:(c0 + CH) * PW],
            )
            nd += 1

        # remap gather: DRAM -> [g, (j, t, w)] tiles, one DMA per plane j
        for j in range(4):
            dq(nd).dma_start(
                out=rhsbig[:, j * TPW:(j + 1) * TPW].rearrange(
                    "g (t w) -> g t w", t=T
                ),
                in_=bass.AP(
                    tensor=sco_d.tensor,
                    offset=sco_d.offset + (32 * j) * (G * PW),
                    ap=[[PW, G], [G * PW, T], [1, PW]],
                ),
            )
            nd += 1

        # ---------- phase 3: scale matmul + correction + diag out ----------
        for r in range(R):
            scP = ps_sc.tile([128, 512], f32, tag="sc", name=f"scP{rep}_{r}")
            cP = ps_c.tile([128, T], f32, tag="c", name=f"cP{rep}_{r}")
            for j in range(4):
                rhs_ap = rhsbig[:, :].rearrange(
                    "g (j t w) -> g j w t", j=4, t=T
                )[:, j, 32 * r:32 * r + 32, :]
                nc.tensor.matmul(
                    scP[32 * j:32 * (j + 1), :],
                    sp16[:, j * PW + 32 * r: j * PW + 32 * r + 32],
                    rhs_ap,
                    start=True, stop=True, tile_position=(0, 32 * j),
                )
                nc.tensor.matmul(
                    cP[32 * j:32 * (j + 1), :],
                    szb[:, j * PW + 32 * r: j * PW + 32 * r + 32],
                    xsum_aug[:],
                    start=True, stop=True, tile_position=(0, 32 * j),
                )
            scS = smallp.tile([128, 512], f16, tag="scS")
            if r % 2 == 0:
                nc.vector.tensor_copy(scS[:], scP[:])
            else:
                nc.scalar.copy(scS[:], scP[:])
            dq(nd).dma_start(out=scr_d[r], in_=scS[:])
            nd += 1
            diagbuf = smallp.tile([128, T], f16, tag="diagbuf")
            diag_src = bass.AP(
                tensor=scr_d.tensor,
                offset=scr_d.offset + r * 128 * 512,
                ap=[[32 * 512, 4], [512 + 16, 32], [1, T]],
            )
            dq(nd).dma_start(out=diagbuf[:], in_=diag_src)
            nd += 1
            oT = smallp.tile([128, T], f32, tag="oT")
            nc.vector.scalar_tensor_tensor(
                out=oT[:], in0=diagbuf[:], scalar=0.0, in1=cP[:],
                op0=AluOpType.bypass, op1=AluOpType.subtract,
            )
            dq(nd).dma_start(
                out=out_d[:].rearrange("(j w) t -> j w t", j=4)[
                    :, 32 * r:32 * r + 32, :
                ],
                in_=oT[:],
            )
            nd += 1
    return nc


# ---------------------------------------------------------------- entry

_CACHE = {}


def _get_nc(cfg):
    key = (cfg.K, cfg.NPAD, cfg.T)
    if key not in _CACHE:
        nc = bacc.Bacc(num_devices=N_CORES)
        build_kernel(nc, cfg)
        nc.compile()
        _CACHE[key] = nc
    return _CACHE[key]


def kernel(x, qweight, qzeros, scales, bias):
    cfg = FULL
    in_maps = host_prep(cfg, x, qweight, qzeros, scales, bias)
    nc = _get_nc(cfg)
    res = run_bass_kernel_spmd(nc, in_maps, core_ids=list(range(N_CORES)))
    return host_gather(cfg, res.results)


# revision 25
# speedup vs baseline: 14.7902x; 1.1196x over previous
"""GPTQ-style 4-bit quantized linear (x @ dequant(qweight) + bias) on 8 TRN2 cores.

Column-parallel: N=11008 sharded across 8 cores (1376 each, padded to
1408 = 4 planes x 352). Host prep is bit-layout repacking only:
 - nibbles repacked along N (8 per int32 word) so a word unpacks to
   same-k different-n values, then k-tiled to [128, G*NW] (partition-major)
 - x transposed/tiled to [128, G*16]
 - scales permuted to the plane-major column order the unpack produces

Device kernel per core:
 1. Unpack: fused (and, or) tensor_scalar ops -> fp16 planes in place:
    value = 1024 + E*q (E in {1,16}), exponent 0x6400. The >>8 shift is a
    1-byte-offset DMA copy. G/CH chunks x (1 shift-DMA + 4 plane ops).
 2. Mains (fp16): lhsT = xT_g [128,32] (tokens padded), rhs = plane
    [128,352], 4 planes col-tiled -> per-group partials; PSUM not
    accumulated across groups (GPTQ group scales differ).
 3. Evac: per-group tensor_scalar/ACT-activation subtracts the 1024-offset
    (per-partition bias = -1024*xsum) and writes fp16; a single merged DMA
    per group remaps [(32j+t), w] -> [g, (j,t,w)].
 4. Scale matmul (fp16) contracts groups: lhsT = (s/E)-window [32,32],
    rhs free order (w, t) so the diagonal = contiguous 16-element runs.
 5. Correction matmul C^T[n,t] = sum_g SZ[g,n]*xsum[t,g] - bias[n].
 6. scP -> SBUF fp16 -> DRAM scratch -> flat diag DMA -> subtract -> out.

Math: out[t,n] = sum_g s'[g,n]*S'_g[t,n] - (sum_g SZ[g,n]*xsum[t,g] - bias)
  S'_g = sum_{k in g} x_k*E*q (offset removed at evac), s' = fp16(s/E),
  SZ = s'*zenc + (s - 1024*s'), zenc = fp16-encoded zeros plane.
"""

import numpy as np
from contextlib import ExitStack

import concourse.bass as bass
import concourse.tile as tile
from concourse import mybir, bacc
from concourse.alu_op_type import AluOpType
from concourse.bass_utils import run_bass_kernel_spmd
from concourse.masks import make_identity

MASK_LO = 0x000F000F
MASK_HI = 0x00F000F0
EXP16 = 0x64006400
N_CORES = 8
GROUPSIZE = 128


class Cfg:
    def __init__(self, K=4096, N_shard=1376, T=16, chunk=8):
        self.K = K
        self.G = K // GROUPSIZE
        self.T = T
        self.N_shard = N_shard
        per_plane = -(-N_shard // 8) * 2
        self.PW = -(-per_plane // 32) * 32
        self.NPAD = 4 * self.PW
        self.NW = self.NPAD // 8
        self.R = self.PW // 32
        self.CH = min(chunk, self.G)      # groups per unpack chunk
        assert self.G % self.CH == 0 and N_shard % 8 == 0


FULL = Cfg()

# ---------------------------------------------------------------- host prep


def _unpack_rows(packed, rows):
    w = packed.view(np.uint32)
    out = np.empty((rows, packed.shape[1]), dtype=np.uint8)
    for b in range(8):
        out[b::8] = ((w >> np.uint32(4 * b)) & np.uint32(0xF)).astype(np.uint8)
    return out


def _unpack_cols(packed):
    w = packed.view(np.uint32)
    out = np.empty((w.shape[0], w.shape[1] * 8), dtype=np.uint8)
    for b in range(8):
        out[:, b::8] = ((w >> np.uint32(4 * b)) & np.uint32(0xF)).astype(np.uint8)
    return out


def _pack_cols(nib):
    w = np.zeros((nib.shape[0], nib.shape[1] // 8), dtype=np.uint32)
    for b in range(8):
        w |= nib[:, b::8].astype(np.uint32) << np.uint32(4 * b)
    return w.view(np.int32)


def _perm(cfg):
    p = np.empty(cfg.NPAD, dtype=np.int64)
    m = np.arange(cfg.PW // 2)
    for j in range(4):
        for h in range(2):
            p[j * cfg.PW + 2 * m + h] = 8 * m + j + 4 * h
    return p


def _escale(cfg):
    e = np.ones(cfg.NPAD, dtype=np.float32)
    e[cfg.PW:2 * cfg.PW] = 16.0
    e[3 * cfg.PW:] = 16.0
    return e


def host_prep(cfg, x, qweight, qzeros, scales, bias):
    nib = _unpack_rows(np.asarray(qweight), cfg.K)
    znib = _unpack_cols(np.asarray(qzeros))
    perm, e = _perm(cfg), _escale(cfg)
    x = np.asarray(x, dtype=np.float32)
    # xt[p, g*T+t] = x[t, g*128+p]
    xt = np.ascontiguousarray(
        x.reshape(cfg.T, cfg.G, 128).transpose(2, 1, 0).reshape(128, cfg.G * cfg.T)
    )
    in_maps = []
    for c in range(N_CORES):
        sl = slice(c * cfg.N_shard, (c + 1) * cfg.N_shard)
        nib_s = np.zeros((cfg.K, cfg.NPAD), dtype=np.uint8)
        nib_s[:, : cfg.N_shard] = nib[:, sl]
        znib_s = np.zeros((cfg.G, cfg.NPAD), dtype=np.uint8)
        znib_s[:, : cfg.N_shard] = znib[:, sl]
        s_s = np.zeros((cfg.G, cfg.NPAD), dtype=np.float32)
        s_s[:, : cfg.N_shard] = scales[:, sl]
        b_s = np.zeros(cfg.NPAD, dtype=np.float32)
        b_s[: cfg.N_shard] = bias[sl]
        qw2 = _pack_cols(nib_s)  # [K, NW]
        # partition-major tiling: qwt[p, g*NW+m] = qw2[g*128+p, m]
        # chunk-major: qwt row-block for chunk c is fully contiguous in DRAM
        qwt = np.ascontiguousarray(
            qw2.reshape(cfg.G // cfg.CH, cfg.CH, 128, cfg.NW)
            .transpose(0, 2, 1, 3)
            .reshape(cfg.G // cfg.CH, 128, cfg.CH * cfg.NW)
        ).reshape(128 * (cfg.G // cfg.CH), cfg.CH * cfg.NW)
        s_p = s_s[:, perm]
        spv = (s_p / e[None, :]).astype(np.float16)
        in_maps.append(
            {
                "qw": qwt,
                "qz": _pack_cols(znib_s),
                "sp": spv,
                "s2": (s_p - 1024.0 * spv.astype(np.float64)).astype(np.float32),
                "biasp": (-b_s[perm]).astype(np.float32),
                "xt": xt,
            }
        )
    return in_maps


def host_gather(cfg, results):
    perm = _perm(cfg)
    valid = perm < cfg.N_shard
    out = np.empty((cfg.T, cfg.N_shard * N_CORES), dtype=np.float32)
    for c in range(N_CORES):
        oT = results[c]["outT"]
        shard = np.empty((cfg.T, cfg.N_shard), dtype=np.float32)
        shard[:, perm[valid]] = oT[valid].T
        out[:, c * cfg.N_shard:(c + 1) * cfg.N_shard] = shard
    return out


# ---------------------------------------------------------------- device kernel


def build_kernel(nc, cfg, reps=1):
    f32, f16, i32 = mybir.dt.float32, mybir.dt.float16, mybir.dt.int32
    u8 = mybir.dt.uint8
    G, T, PW, NW, R, CH = cfg.G, cfg.T, cfg.PW, cfg.NW, cfg.R, cfg.CH
    NPAD = cfg.NPAD
    TPW = T * PW
    # mains strips of width SW (<=512) covering NPAD; the last strip
    # overlaps its predecessor so every PSUM column is written exactly
    SW = min(512, NPAD)
    SW2 = SW
    NSTR = -(-NPAD // SW)
    STRIPS = [(i * SW, 0) for i in range(NSTR - 1)]
    STRIPS.append((NPAD - SW, SW - (NPAD - (NSTR - 1) * SW)))
    PSTR = 32 * NSTR

    qw_d = nc.declare_dram_parameter("qw", [128 * (G // CH), CH * NW], i32, isOutput=False)
    qz_d = nc.declare_dram_parameter("qz", [G, NW], i32, isOutput=False)
    sp_d = nc.declare_dram_parameter("sp", [G, NPAD], f16, isOutput=False)
    s2_d = nc.declare_dram_parameter("s2", [G, NPAD], f32, isOutput=False)
    bias_d = nc.declare_dram_parameter("biasp", [NPAD], f32, isOutput=False)
    xt_d = nc.declare_dram_parameter("xt", [128, G * T], f32, isOutput=False)
    out_d = nc.declare_dram_parameter("outT", [NPAD, T], f32, isOutput=True)
    scr_d = nc.dram_tensor("scratch", [R, 128, 512], f16).ap()
    sco_d = nc.dram_tensor("scopy_dram", [16 * NSTR, G * SW2], f16).ap()

    dmae = [nc.sync, nc.scalar]  # the two HWDGE rings

    def dq(i):
        return dmae[i % len(dmae)]

    with tile.TileContext(nc) as tc:
      for rep in range(reps):
       with ExitStack() as ctx:
        singles = ctx.enter_context(tc.tile_pool(name=f"singles{rep}", bufs=1))
        qwp = ctx.enter_context(tc.tile_pool(name=f"qwp{rep}", bufs=2))
        encp = ctx.enter_context(tc.tile_pool(name=f"encp{rep}", bufs=2))
        smallp = ctx.enter_context(tc.tile_pool(name=f"smallp{rep}", bufs=2))
        ps_main = ctx.enter_context(tc.tile_pool(name=f"ps_main{rep}", bufs=1, space="PSUM"))
        ps_sc = ctx.enter_context(tc.tile_pool(name=f"ps_sc{rep}", bufs=2, space="PSUM"))
        ps_c = ctx.enter_context(tc.tile_pool(name=f"ps_c{rep}", bufs=2, space="PSUM"))

        # ---------- phase 0: x prep ----------
        xf = singles.tile([128, G * T], f32)
        nc.sync.dma_start(out=xf[:], in_=xt_d[:])
        # tokens padded to 32 per group so matmuls write full PSUM strips
        xT = singles.tile([128, G * 32], f16)
        nc.vector.memset(xT[:], 0.0)
        nc.vector.tensor_copy(
            xT[:].rearrange("p (g t) -> p g t", g=G)[:, :, 0:T], xf[:]
        )
        ones16 = singles.tile([128, 1], f16)
        nc.vector.memset(ones16[:], 1.0)

        xsumP = ps_sc.tile([T, G], f32, tag="sc")
        for g in range(G):
            nc.tensor.matmul(
                xsumP[:, g:g + 1], xT[:, g * 32:g * 32 + T], ones16[:],
                start=True, stop=True,
            )
        xsum_s = singles.tile([T, G], f32)
        nc.scalar.copy(xsum_s[:], xsumP[:])
        ident = singles.tile([T, T], f32)
        make_identity(nc, ident[:])
        xsT_P = ps_sc.tile([G, T], f32, tag="sc")
        nc.tensor.transpose(xsT_P[:], xsum_s[:], ident[:])
        xsum_aug = singles.tile([G + 1, T], f32)
        nc.vector.memset(xsum_aug[:], 1.0)
        nc.scalar.copy(xsum_aug[:G, :], xsT_P[:])
        # offv[32j+t, g] = -1024 * xsum[t, g]
        offv = singles.tile([128, G], f32)
        nc.vector.memset(offv[:], 0.0)
        for j in range(4):
            nc.scalar.mul(offv[32 * j:32 * j + T, :], xsumP[:], -1024.0)

        # ---------- phase 1: scales / zeros prep ----------
        sp16 = singles.tile([G, NPAD], f16)
        nc.sync.dma_start(out=sp16[:], in_=sp_d[:])
        sp_s = singles.tile([G, NPAD], f32)
        nc.vector.tensor_copy(sp_s[:], sp16[:])
        s_s = singles.tile([G, NPAD], f32)
        nc.sync.dma_start(out=s_s[:], in_=s2_d[:])

        qz_t = singles.tile([G, NW], i32)
        nc.sync.dma_start(out=qz_t[:], in_=qz_d[:])
        zs = singles.tile([G, NW], i32)
        nc.vector.tensor_scalar(
            out=zs[:], in0=qz_t[:], scalar1=8, scalar2=None,
            op0=AluOpType.logical_shift_right,
        )
        zenc = singles.tile([G, NPAD], f16)
        zi = zenc[:].bitcast(i32)
        for j, (src, mask) in enumerate(
            [(qz_t, MASK_LO), (qz_t, MASK_HI), (zs, MASK_LO), (zs, MASK_HI)]
        ):
            nc.vector.tensor_scalar(
                out=zi[:, j * NW:(j + 1) * NW], in0=src[:],
                scalar1=mask, scalar2=EXP16,
                op0=AluOpType.bitwise_and, op1=AluOpType.bitwise_or,
            )
        zf = singles.tile([G, NPAD], f32)
        nc.vector.tensor_copy(zf[:], zenc[:])
        szb = singles.tile([G + 1, NPAD], f32)
        nc.vector.tensor_tensor(szb[:G, :], zf[:], sp_s[:], AluOpType.mult)
        nc.vector.tensor_tensor(szb[:G, :], szb[:G, :], s_s[:], AluOpType.add)
        nc.sync.dma_start(out=szb[G:G + 1, :], in_=bias_d[None, :])

        # ---------- phase 2: unpack + mains + evac ----------
        rhsbig = singles.tile([G, T * NPAD], f16)
        scopy = singles.tile([128, G * SW2], f16)
        nd = 0  # DMA ring round-robin counter
        for c0 in range(0, G, CH):
            wt = qwp.tile([128, CH * NW], i32, tag="wt")
            nc.sync.dma_start(
                out=wt[:], in_=qw_d[(c0 // CH) * 128:(c0 // CH + 1) * 128, :]
            )
            ws = qwp.tile([128, CH * NW], i32, tag="ws")
            nc.vector.tensor_scalar(
                out=ws[:], in0=wt[:], scalar1=8, scalar2=None,
                op0=AluOpType.logical_shift_right,
            )
            enc = encp.tile([128, CH * NPAD], f16, tag="enc")
            ei = enc[:].bitcast(i32)
            for j, (src, mask) in enumerate(
                [(wt, MASK_LO), (wt, MASK_HI), (ws, MASK_LO), (ws, MASK_HI)]
            ):
                nc.vector.tensor_scalar(
                    out=ei[:].rearrange("p (g w) -> p g w", g=CH)[
                        :, :, j * NW:(j + 1) * NW
                    ],
                    in0=src[:],
                    scalar1=mask, scalar2=EXP16,
                    op0=AluOpType.bitwise_and, op1=AluOpType.bitwise_or,
                )
            for gg in range(CH):
                g = c0 + gg
                mainP = ps_main.tile(
                    [128, 512], f32, tag=f"m{g % 4}", name=f"mainP{rep}_{g % 4}"
                )
                for sidx, (c_lo, _) in enumerate(STRIPS):
                    nc.tensor.matmul(
                        mainP[32 * sidx:32 * (sidx + 1), 0:SW],
                        xT[:, g * 32:(g + 1) * 32],
                        enc[:, gg * NPAD + c_lo: gg * NPAD + c_lo + SW],
                        start=True, stop=True, tile_position=(0, 32 * sidx),
                    )
                # evac: subtract offset, fp16 (alternate DVE / ACT)
                if g % 2 == 0:
                    nc.vector.tensor_scalar(
                        out=scopy[0:PSTR, g * SW2:(g + 1) * SW2],
                        in0=mainP[0:PSTR, 0:SW],
                        scalar1=offv[0:PSTR, g:g + 1], scalar2=None,
                        op0=AluOpType.add,
                    )
                else:
                    nc.scalar.activation(
                        scopy[0:PSTR, g * SW2:(g + 1) * SW2], mainP[0:PSTR, 0:SW],
                        mybir.ActivationFunctionType.Identity,
                        bias=offv[0:PSTR, g:g + 1], scale=1.0,
                    )

            # chunk's evac window -> DRAM mirror (contiguous per partition)
            for sidx in range(NSTR):
                dq(nd).dma_start(
                    out=sco_d[16 * sidx:16 * (sidx + 1),
                              c0 * SW2:(c0 + CH) * SW2],
                    in_=scopy[32 * sidx:32 * sidx + T,
                              c0 * SW2:(c0 + CH) * SW2],
                )
                nd += 1

        # remap gather: DRAM -> [g, (t, nflat)] tiles, one DMA per strip
        for sidx, (c_lo, f_lo) in enumerate(STRIPS):
            width = SW - f_lo
            dq(nd).dma_start(
                out=rhsbig[:, :].rearrange("g (t n) -> g t n", t=T)[
                    :, :, c_lo + f_lo: c_lo + f_lo + width
                ],
                in_=bass.AP(
                    tensor=sco_d.tensor,
                    offset=sco_d.offset + (16 * sidx) * (G * SW2) + f_lo,
                    ap=[[SW2, G], [G * SW2, T], [1, width]],
                ),
            )
            nd += 1

        # ---------- phase 3: scale matmul + correction + diag out ----------
        for u in range(R):
            scP = ps_sc.tile([128, 512], f32, tag="sc", name=f"scP{rep}_{u}")
            cP = ps_c.tile([128, T], f32, tag="c", name=f"cP{rep}_{u}")
            for v in range(4):
                w0 = 128 * u + 32 * v
                rhs_ap = rhsbig[:, :].rearrange("g (t n) -> g n t", t=T)[
                    :, w0:w0 + 32, :
                ]
                nc.tensor.matmul(
                    scP[32 * v:32 * (v + 1), :],
                    sp16[:, w0:w0 + 32],
                    rhs_ap,
                    start=True, stop=True, tile_position=(0, 32 * v),
                )
            nc.tensor.matmul(
                cP[:, :],
                szb[:, 128 * u:128 * (u + 1)],
                xsum_aug[:],
                start=True, stop=True,
            )
            scS = smallp.tile([128, 512], f16, tag="scS")
            if u % 2 == 0:
                nc.vector.tensor_copy(scS[:], scP[:])
            else:
                nc.scalar.copy(scS[:], scP[:])
            dq(nd).dma_start(out=scr_d[u], in_=scS[:])
            nd += 1
            diagbuf = smallp.tile([128, T], f16, tag="diagbuf")
            diag_src = bass.AP(
                tensor=scr_d.tensor,
                offset=scr_d.offset + u * 128 * 512,
                ap=[[32 * 512, 4], [512 + 16, 32], [1, T]],
            )
            dq(nd).dma_start(out=diagbuf[:], in_=diag_src)
            nd += 1
            oT = smallp.tile([128, T], f32, tag="oT")
            nc.vector.scalar_tensor_tensor(
                out=oT[:], in0=diagbuf[:], scalar=0.0, in1=cP[:],
                op0=AluOpType.bypass, op1=AluOpType.subtract,
            )
            dq(nd).dma_start(out=out_d[128 * u:128 * (u + 1), :], in_=oT[:])
            nd += 1
    return nc


# ---------------------------------------------------------------- entry

_CACHE = {}


def _get_nc(cfg):
    key = (cfg.K, cfg.NPAD, cfg.T)
    if key not in _CACHE:
        nc = bacc.Bacc(num_devices=N_CORES)
        build_kernel(nc, cfg)
        nc.compile()
        _CACHE[key] = nc
    return _CACHE[key]


def kernel(x, qweight, qzeros, scales, bias):
    cfg = FULL
    in_maps = host_prep(cfg, x, qweight, qzeros, scales, bias)
    nc = _get_nc(cfg)
    res = run_bass_kernel_spmd(nc, in_maps, core_ids=list(range(N_CORES)))
    return host_gather(cfg, res.results)


# revision 26
# speedup vs baseline: 22.2134x; 1.5019x over previous
"""GPTQ-style 4-bit quantized linear (x @ dequant(qweight) + bias) on 8 TRN2 cores.

Column-parallel: N=11008 sharded across 8 cores (1376 each, padded to
1408 = 4 planes x 352). Host prep is bit-layout repacking only:
 - nibbles repacked along N (8 per int32 word) so a word unpacks to
   same-k different-n values, then k-tiled to [128, G*NW] (partition-major)
 - x transposed/tiled to [128, G*16]
 - scales permuted to the plane-major column order the unpack produces

Device kernel per core:
 1. Unpack: fused (and, or) tensor_scalar ops -> fp16 planes in place:
    value = 1024 + E*q (E in {1,16}), exponent 0x6400. The >>8 shift is a
    1-byte-offset DMA copy. G/CH chunks x (1 shift-DMA + 4 plane ops).
 2. Mains (fp16): lhsT = xT_g [128,32] (tokens padded), rhs = plane
    [128,352], 4 planes col-tiled -> per-group partials; PSUM not
    accumulated across groups (GPTQ group scales differ).
 3. Evac: per-group tensor_scalar/ACT-activation subtracts the 1024-offset
    (per-partition bias = -1024*xsum) and writes fp16; a single merged DMA
    per group remaps [(32j+t), w] -> [g, (j,t,w)].
 4. Scale matmul (fp16) contracts groups: lhsT = (s/E)-window [32,32],
    rhs free order (w, t) so the diagonal = contiguous 16-element runs.
 5. Correction matmul C^T[n,t] = sum_g SZ[g,n]*xsum[t,g] - bias[n].
 6. scP -> SBUF fp16 -> DRAM scratch -> flat diag DMA -> subtract -> out.

Math: out[t,n] = sum_g s'[g,n]*S'_g[t,n] - (sum_g SZ[g,n]*xsum[t,g] - bias)
  S'_g = sum_{k in g} x_k*E*q (offset removed at evac), s' = fp16(s/E),
  SZ = s'*zenc + (s - 1024*s'), zenc = fp16-encoded zeros plane.
"""

import numpy as np
from contextlib import ExitStack

import concourse.bass as bass
import concourse.tile as tile
from concourse import mybir, bacc
from concourse.alu_op_type import AluOpType
from concourse.bass_utils import run_bass_kernel_spmd
from concourse.masks import make_identity

MASK_LO = 0x000F000F
MASK_HI = 0x00F000F0
EXP16 = 0x64006400
N_CORES = 8
GROUPSIZE = 128


class Cfg:
    def __init__(self, K=4096, N_shard=1376, T=16, chunk=8):
        self.K = K
        self.G = K // GROUPSIZE
        self.T = T
        self.N_shard = N_shard
        per_plane = -(-N_shard // 8) * 2
        self.PW = -(-per_plane // 32) * 32
        self.NPAD = 4 * self.PW
        self.NW = self.NPAD // 8
        self.R = self.PW // 32
        self.CH = min(chunk, self.G)      # groups per unpack chunk
        assert self.G % self.CH == 0 and N_shard % 8 == 0


FULL = Cfg()

# ---------------------------------------------------------------- host prep


def _unpack_rows(packed, rows):
    w = packed.view(np.uint32)
    out = np.empty((rows, packed.shape[1]), dtype=np.uint8)
    for b in range(8):
        out[b::8] = ((w >> np.uint32(4 * b)) & np.uint32(0xF)).astype(np.uint8)
    return out


def _unpack_cols(packed):
    w = packed.view(np.uint32)
    out = np.empty((w.shape[0], w.shape[1] * 8), dtype=np.uint8)
    for b in range(8):
        out[:, b::8] = ((w >> np.uint32(4 * b)) & np.uint32(0xF)).astype(np.uint8)
    return out


def _pack_cols(nib):
    w = np.zeros((nib.shape[0], nib.shape[1] // 8), dtype=np.uint32)
    for b in range(8):
        w |= nib[:, b::8].astype(np.uint32) << np.uint32(4 * b)
    return w.view(np.int32)


def _perm(cfg):
    p = np.empty(cfg.NPAD, dtype=np.int64)
    m = np.arange(cfg.PW // 2)
    for j in range(4):
        for h in range(2):
            p[j * cfg.PW + 2 * m + h] = 8 * m + j + 4 * h
    return p


def _escale(cfg):
    e = np.ones(cfg.NPAD, dtype=np.float32)
    e[cfg.PW:2 * cfg.PW] = 16.0
    e[3 * cfg.PW:] = 16.0
    return e


def host_prep(cfg, x, qweight, qzeros, scales, bias):
    nib = _unpack_rows(np.asarray(qweight), cfg.K)
    znib = _unpack_cols(np.asarray(qzeros))
    perm, e = _perm(cfg), _escale(cfg)
    x = np.asarray(x, dtype=np.float32)
    # xt[p, g*T+t] = x[t, g*128+p]
    xt = np.ascontiguousarray(
        x.reshape(cfg.T, cfg.G, 128).transpose(2, 1, 0).reshape(128, cfg.G * cfg.T)
    )
    in_maps = []
    for c in range(N_CORES):
        sl = slice(c * cfg.N_shard, (c + 1) * cfg.N_shard)
        nib_s = np.zeros((cfg.K, cfg.NPAD), dtype=np.uint8)
        nib_s[:, : cfg.N_shard] = nib[:, sl]
        znib_s = np.zeros((cfg.G, cfg.NPAD), dtype=np.uint8)
        znib_s[:, : cfg.N_shard] = znib[:, sl]
        s_s = np.zeros((cfg.G, cfg.NPAD), dtype=np.float32)
        s_s[:, : cfg.N_shard] = scales[:, sl]
        b_s = np.zeros(cfg.NPAD, dtype=np.float32)
        b_s[: cfg.N_shard] = bias[sl]
        qw2 = _pack_cols(nib_s)  # [K, NW]
        # partition-major tiling: qwt[p, g*NW+m] = qw2[g*128+p, m]
        # chunk-major: qwt row-block for chunk c is fully contiguous in DRAM
        qwt = np.ascontiguousarray(
            qw2.reshape(cfg.G // cfg.CH, cfg.CH, 128, cfg.NW)
            .transpose(0, 2, 1, 3)
            .reshape(cfg.G // cfg.CH, 128, cfg.CH * cfg.NW)
        ).reshape(128 * (cfg.G // cfg.CH), cfg.CH * cfg.NW)
        s_p = s_s[:, perm]
        spv = (s_p / e[None, :]).astype(np.float16)
        in_maps.append(
            {
                "qw": qwt,
                "qz": _pack_cols(znib_s),
                "sp": spv,
                "s2": (s_p - 1024.0 * spv.astype(np.float64)).astype(np.float32),
                "biasp": (-b_s[perm]).astype(np.float32),
                "xt": xt,
            }
        )
    return in_maps


def host_gather(cfg, results):
    perm = _perm(cfg)
    valid = perm < cfg.N_shard
    out = np.empty((cfg.T, cfg.N_shard * N_CORES), dtype=np.float32)
    for c in range(N_CORES):
        oT = results[c]["outT"]
        shard = np.empty((cfg.T, cfg.N_shard), dtype=np.float32)
        shard[:, perm[valid]] = oT[valid].T
        out[:, c * cfg.N_shard:(c + 1) * cfg.N_shard] = shard
    return out


# ---------------------------------------------------------------- device kernel


def build_kernel(nc, cfg, reps=1):
    f32, f16, i32 = mybir.dt.float32, mybir.dt.float16, mybir.dt.int32
    u8 = mybir.dt.uint8
    G, T, PW, NW, R, CH = cfg.G, cfg.T, cfg.PW, cfg.NW, cfg.R, cfg.CH
    NPAD = cfg.NPAD
    TPW = T * PW
    # mains strips of width SW (<=512) covering NPAD; the last strip
    # overlaps its predecessor so every PSUM column is written exactly
    SW = min(512, NPAD)
    SW2 = SW
    NSTR = -(-NPAD // SW)
    STRIPS = [(i * SW, 0) for i in range(NSTR - 1)]
    STRIPS.append((NPAD - SW, SW - (NPAD - (NSTR - 1) * SW)))
    PSTR = 32 * NSTR

    qw_d = nc.declare_dram_parameter("qw", [128 * (G // CH), CH * NW], i32, isOutput=False)
    qz_d = nc.declare_dram_parameter("qz", [G, NW], i32, isOutput=False)
    sp_d = nc.declare_dram_parameter("sp", [G, NPAD], f16, isOutput=False)
    s2_d = nc.declare_dram_parameter("s2", [G, NPAD], f32, isOutput=False)
    bias_d = nc.declare_dram_parameter("biasp", [NPAD], f32, isOutput=False)
    xt_d = nc.declare_dram_parameter("xt", [128, G * T], f32, isOutput=False)
    out_d = nc.declare_dram_parameter("outT", [NPAD, T], f32, isOutput=True)
    scr_d = nc.dram_tensor("scratch", [R, 128, 512], f16).ap()
    sco_d = nc.dram_tensor("scopy_dram", [PSTR, G * SW2], f16).ap()

    dmae = [nc.sync, nc.scalar]  # the two HWDGE rings

    def dq(i):
        return dmae[i % len(dmae)]

    with tile.TileContext(nc) as tc:
      for rep in range(reps):
       with ExitStack() as ctx:
        singles = ctx.enter_context(tc.tile_pool(name=f"singles{rep}", bufs=1))
        qwp = ctx.enter_context(tc.tile_pool(name=f"qwp{rep}", bufs=2))
        encp = ctx.enter_context(tc.tile_pool(name=f"encp{rep}", bufs=2))
        smallp = ctx.enter_context(tc.tile_pool(name=f"smallp{rep}", bufs=2))
        ps_main = ctx.enter_context(tc.tile_pool(name=f"ps_main{rep}", bufs=1, space="PSUM"))
        ps_sc = ctx.enter_context(tc.tile_pool(name=f"ps_sc{rep}", bufs=2, space="PSUM"))
        ps_c = ctx.enter_context(tc.tile_pool(name=f"ps_c{rep}", bufs=2, space="PSUM"))

        # ---------- phase 0: x prep ----------
        xf = singles.tile([128, G * T], f32)
        nc.sync.dma_start(out=xf[:], in_=xt_d[:])
        # tokens padded to 32 per group so matmuls write full PSUM strips
        xT = singles.tile([128, G * 32], f16)
        nc.vector.memset(xT[:], 0.0)
        nc.vector.tensor_copy(
            xT[:].rearrange("p (g t) -> p g t", g=G)[:, :, 0:T], xf[:]
        )
        ones16 = singles.tile([128, 1], f16)
        nc.vector.memset(ones16[:], 1.0)

        xsumP = ps_sc.tile([T, G], f32, tag="sc")
        for g in range(G):
            nc.tensor.matmul(
                xsumP[:, g:g + 1], xT[:, g * 32:g * 32 + T], ones16[:],
                start=True, stop=True,
            )
        xsum_s = singles.tile([T, G], f32)
        nc.scalar.copy(xsum_s[:], xsumP[:])
        ident = singles.tile([T, T], f32)
        make_identity(nc, ident[:])
        xsT_P = ps_sc.tile([G, T], f32, tag="sc")
        nc.tensor.transpose(xsT_P[:], xsum_s[:], ident[:])
        xsum_aug = singles.tile([G + 1, T], f32)
        nc.vector.memset(xsum_aug[:], 1.0)
        nc.scalar.copy(xsum_aug[:G, :], xsT_P[:])
        # offv[32j+t, g] = -1024 * xsum[t, g]
        offv = singles.tile([128, G], f32)
        nc.vector.memset(offv[:], 0.0)
        for j in range(4):
            nc.scalar.mul(offv[32 * j:32 * j + T, :], xsumP[:], -1024.0)

        # ---------- phase 1: scales / zeros prep ----------
        sp16 = singles.tile([G, NPAD], f16)
        nc.sync.dma_start(out=sp16[:], in_=sp_d[:])
        sp_s = singles.tile([G, NPAD], f32)
        nc.vector.tensor_copy(sp_s[:], sp16[:])
        s_s = singles.tile([G, NPAD], f32)
        nc.sync.dma_start(out=s_s[:], in_=s2_d[:])

        qz_t = singles.tile([G, NW], i32)
        nc.sync.dma_start(out=qz_t[:], in_=qz_d[:])
        zs = singles.tile([G, NW], i32)
        nc.vector.tensor_scalar(
            out=zs[:], in0=qz_t[:], scalar1=8, scalar2=None,
            op0=AluOpType.logical_shift_right,
        )
        zenc = singles.tile([G, NPAD], f16)
        zi = zenc[:].bitcast(i32)
        for j, (src, mask) in enumerate(
            [(qz_t, MASK_LO), (qz_t, MASK_HI), (zs, MASK_LO), (zs, MASK_HI)]
        ):
            nc.vector.tensor_scalar(
                out=zi[:, j * NW:(j + 1) * NW], in0=src[:],
                scalar1=mask, scalar2=EXP16,
                op0=AluOpType.bitwise_and, op1=AluOpType.bitwise_or,
            )
        zf = singles.tile([G, NPAD], f32)
        nc.vector.tensor_copy(zf[:], zenc[:])
        szb = singles.tile([G + 1, NPAD], f32)
        nc.vector.tensor_tensor(szb[:G, :], zf[:], sp_s[:], AluOpType.mult)
        nc.vector.tensor_tensor(szb[:G, :], szb[:G, :], s_s[:], AluOpType.add)
        nc.sync.dma_start(out=szb[G:G + 1, :], in_=bias_d[None, :])

        # ---------- phase 2: unpack + mains + evac ----------
        rhsbig = singles.tile([G, T * NPAD], f16)
        scopy = singles.tile([128, G * SW2], f16)
        nd = 0  # DMA ring round-robin counter
        for c0 in range(0, G, CH):
            wt = qwp.tile([128, CH * NW], i32, tag="wt")
            dq(nd).dma_start(
                out=wt[:], in_=qw_d[(c0 // CH) * 128:(c0 // CH + 1) * 128, :]
            )
            nd += 1
            ws = qwp.tile([128, CH * NW], i32, tag="ws")
            nc.vector.tensor_scalar(
                out=ws[:], in0=wt[:], scalar1=8, scalar2=None,
                op0=AluOpType.logical_shift_right,
            )
            enc = encp.tile([128, CH * NPAD], f16, tag="enc")
            ei = enc[:].bitcast(i32)
            for j, (src, mask) in enumerate(
                [(wt, MASK_LO), (wt, MASK_HI), (ws, MASK_LO), (ws, MASK_HI)]
            ):
                nc.vector.tensor_scalar(
                    out=ei[:].rearrange("p (g w) -> p g w", g=CH)[
                        :, :, j * NW:(j + 1) * NW
                    ],
                    in0=src[:],
                    scalar1=mask, scalar2=EXP16,
                    op0=AluOpType.bitwise_and, op1=AluOpType.bitwise_or,
                )
            for gg in range(CH):
                g = c0 + gg
                mainP = ps_main.tile(
                    [128, 512], f32, tag=f"m{g % 4}", name=f"mainP{rep}_{g % 4}"
                )
                for sidx, (c_lo, _) in enumerate(STRIPS):
                    nc.tensor.matmul(
                        mainP[32 * sidx:32 * (sidx + 1), 0:SW],
                        xT[:, g * 32:(g + 1) * 32],
                        enc[:, gg * NPAD + c_lo: gg * NPAD + c_lo + SW],
                        start=True, stop=True, tile_position=(0, 32 * sidx),
                    )
                # evac: subtract offset, fp16 (alternate DVE / ACT)
                if g % 2 == 0:
                    nc.vector.tensor_scalar(
                        out=scopy[0:PSTR, g * SW2:(g + 1) * SW2],
                        in0=mainP[0:PSTR, 0:SW],
                        scalar1=offv[0:PSTR, g:g + 1], scalar2=None,
                        op0=AluOpType.add,
                    )
                else:
                    nc.scalar.activation(
                        scopy[0:PSTR, g * SW2:(g + 1) * SW2], mainP[0:PSTR, 0:SW],
                        mybir.ActivationFunctionType.Identity,
                        bias=offv[0:PSTR, g:g + 1], scale=1.0,
                    )

            # chunk's evac window -> DRAM mirror (contiguous per partition)
            dq(nd).dma_start(
                out=sco_d[:, c0 * SW2:(c0 + CH) * SW2],
                in_=scopy[0:PSTR, c0 * SW2:(c0 + CH) * SW2],
            )
            nd += 1

        # remap gather: DRAM -> [g, (t, nflat)] tiles, one DMA per strip
        for sidx, (c_lo, f_lo) in enumerate(STRIPS):
            width = SW - f_lo
            dq(nd).dma_start(
                out=rhsbig[:, :].rearrange("g (t n) -> g t n", t=T)[
                    :, :, c_lo + f_lo: c_lo + f_lo + width
                ],
                in_=bass.AP(
                    tensor=sco_d.tensor,
                    offset=sco_d.offset + (32 * sidx) * (G * SW2) + f_lo,
                    ap=[[SW2, G], [G * SW2, T], [1, width]],
                ),
            )
            nd += 1

        # ---------- phase 3: scale matmul + correction + diag out ----------
        for u in range(R):
            scP = ps_sc.tile([128, 512], f32, tag="sc", name=f"scP{rep}_{u}")
            cP = ps_c.tile([128, T], f32, tag="c", name=f"cP{rep}_{u}")
            for v in range(4):
                w0 = 128 * u + 32 * v
                rhs_ap = rhsbig[:, :].rearrange("g (t n) -> g n t", t=T)[
                    :, w0:w0 + 32, :
                ]
                nc.tensor.matmul(
                    scP[32 * v:32 * (v + 1), :],
                    sp16[:, w0:w0 + 32],
                    rhs_ap,
                    start=True, stop=True, tile_position=(0, 32 * v),
                )
            nc.tensor.matmul(
                cP[:, :],
                szb[:, 128 * u:128 * (u + 1)],
                xsum_aug[:],
                start=True, stop=True,
            )
            scS = smallp.tile([128, 512], f16, tag="scS")
            if u % 2 == 0:
                nc.vector.tensor_copy(scS[:], scP[:])
            else:
                nc.scalar.copy(scS[:], scP[:])
            dq(nd).dma_start(out=scr_d[u], in_=scS[:])
            nd += 1
            diagbuf = smallp.tile([128, T], f16, tag="diagbuf")
            diag_src = bass.AP(
                tensor=scr_d.tensor,
                offset=scr_d.offset + u * 128 * 512,
                ap=[[32 * 512, 4], [512 + 16, 32], [1, T]],
            )
            dq(nd).dma_start(out=diagbuf[:], in_=diag_src)
            nd += 1
            oT = smallp.tile([128, T], f32, tag="oT")
            nc.vector.scalar_tensor_tensor(
                out=oT[:], in0=diagbuf[:], scalar=0.0, in1=cP[:],
                op0=AluOpType.bypass, op1=AluOpType.subtract,
            )
            dq(nd).dma_start(out=out_d[128 * u:128 * (u + 1), :], in_=oT[:])
            nd += 1
    return nc


# ---------------------------------------------------------------- entry

_CACHE = {}


def _get_nc(cfg):
    key = (cfg.K, cfg.NPAD, cfg.T)
    if key not in _CACHE:
        nc = bacc.Bacc(num_devices=N_CORES)
        build_kernel(nc, cfg)
        nc.compile()
        _CACHE[key] = nc
    return _CACHE[key]


def kernel(x, qweight, qzeros, scales, bias):
    cfg = FULL
    in_maps = host_prep(cfg, x, qweight, qzeros, scales, bias)
    nc = _get_nc(cfg)
    res = run_bass_kernel_spmd(nc, in_maps, core_ids=list(range(N_CORES)))
    return host_gather(cfg, res.results)


# revision 28
# speedup vs baseline: 28.2892x; 1.2735x over previous
"""GPTQ-style 4-bit quantized linear (x @ dequant(qweight) + bias) on 8 TRN2 cores.

Column-parallel: N=11008 sharded across 8 cores (1376 each, padded to
1408 = 4 planes x 352). Host prep is bit-layout repacking only:
 - nibbles repacked along N (8 per int32 word) so a word unpacks to
   same-k different-n values, then k-tiled to [128, G*NW] (partition-major)
 - x transposed/tiled to [128, G*16]
 - scales permuted to the plane-major column order the unpack produces

Device kernel per core:
 1. Unpack: fused (and, or) tensor_scalar ops -> fp16 planes in place:
    value = 1024 + E*q (E in {1,16}), exponent 0x6400. The >>8 shift is a
    1-byte-offset DMA copy. G/CH chunks x (1 shift-DMA + 4 plane ops).
 2. Mains (fp16): lhsT = xT_g [128,32] (tokens padded), rhs = plane
    [128,352], 4 planes col-tiled -> per-group partials; PSUM not
    accumulated across groups (GPTQ group scales differ).
 3. Evac: per-group tensor_scalar/ACT-activation subtracts the 1024-offset
    (per-partition bias = -1024*xsum) and writes fp16; a single merged DMA
    per group remaps [(32j+t), w] -> [g, (j,t,w)].
 4. Scale matmul (fp16) contracts groups: lhsT = (s/E)-window [32,32],
    rhs free order (w, t) so the diagonal = contiguous 16-element runs.
 5. Correction matmul C^T[n,t] = sum_g SZ[g,n]*xsum[t,g] - bias[n].
 6. scP -> SBUF fp16 -> DRAM scratch -> flat diag DMA -> subtract -> out.

Math: out[t,n] = sum_g s'[g,n]*S'_g[t,n] - (sum_g SZ[g,n]*xsum[t,g] - bias)
  S'_g = sum_{k in g} x_k*E*q (offset removed at evac), s' = fp16(s/E),
  SZ = s'*zenc + (s - 1024*s'), zenc = fp16-encoded zeros plane.
"""

import numpy as np
from contextlib import ExitStack

import concourse.bass as bass
import concourse.tile as tile
from concourse import mybir, bacc
from concourse.alu_op_type import AluOpType
from concourse.bass_utils import run_bass_kernel_spmd
from concourse.masks import make_identity

MASK_LO = 0x000F000F
MASK_HI = 0x00F000F0
EXP16 = 0x64006400
N_CORES = 8
GROUPSIZE = 128


class Cfg:
    def __init__(self, K=4096, N_shard=1376, T=16, chunk=8):
        self.K = K
        self.G = K // GROUPSIZE
        self.T = T
        self.N_shard = N_shard
        per_plane = -(-N_shard // 8) * 2
        self.PW = -(-per_plane // 32) * 32
        self.NPAD = 4 * self.PW
        self.NW = self.NPAD // 8
        self.R = self.PW // 32
        self.CH = min(chunk, self.G)      # groups per unpack chunk
        assert self.G % self.CH == 0 and N_shard % 8 == 0


FULL = Cfg()

# ---------------------------------------------------------------- host prep


def _unpack_rows(packed, rows):
    w = packed.view(np.uint32)
    out = np.empty((rows, packed.shape[1]), dtype=np.uint8)
    for b in range(8):
        out[b::8] = ((w >> np.uint32(4 * b)) & np.uint32(0xF)).astype(np.uint8)
    return out


def _unpack_cols(packed):
    w = packed.view(np.uint32)
    out = np.empty((w.shape[0], w.shape[1] * 8), dtype=np.uint8)
    for b in range(8):
        out[:, b::8] = ((w >> np.uint32(4 * b)) & np.uint32(0xF)).astype(np.uint8)
    return out


def _pack_cols(nib):
    w = np.zeros((nib.shape[0], nib.shape[1] // 8), dtype=np.uint32)
    for b in range(8):
        w |= nib[:, b::8].astype(np.uint32) << np.uint32(4 * b)
    return w.view(np.int32)


def _perm(cfg):
    p = np.empty(cfg.NPAD, dtype=np.int64)
    m = np.arange(cfg.PW // 2)
    for j in range(4):
        for h in range(2):
            p[j * cfg.PW + 2 * m + h] = 8 * m + j + 4 * h
    return p


def _escale(cfg):
    e = np.ones(cfg.NPAD, dtype=np.float32)
    e[cfg.PW:2 * cfg.PW] = 16.0
    e[3 * cfg.PW:] = 16.0
    return e


def host_prep(cfg, x, qweight, qzeros, scales, bias):
    nib = _unpack_rows(np.asarray(qweight), cfg.K)
    znib = _unpack_cols(np.asarray(qzeros))
    perm, e = _perm(cfg), _escale(cfg)
    x = np.asarray(x, dtype=np.float32)
    # xt[p, g*T+t] = x[t, g*128+p]
    xt = np.ascontiguousarray(
        x.reshape(cfg.T, cfg.G, 128).transpose(2, 1, 0).reshape(128, cfg.G * cfg.T)
    )
    in_maps = []
    for c in range(N_CORES):
        sl = slice(c * cfg.N_shard, (c + 1) * cfg.N_shard)
        nib_s = np.zeros((cfg.K, cfg.NPAD), dtype=np.uint8)
        nib_s[:, : cfg.N_shard] = nib[:, sl]
        znib_s = np.zeros((cfg.G, cfg.NPAD), dtype=np.uint8)
        znib_s[:, : cfg.N_shard] = znib[:, sl]
        s_s = np.zeros((cfg.G, cfg.NPAD), dtype=np.float32)
        s_s[:, : cfg.N_shard] = scales[:, sl]
        b_s = np.zeros(cfg.NPAD, dtype=np.float32)
        b_s[: cfg.N_shard] = bias[sl]
        qw2 = _pack_cols(nib_s)  # [K, NW]
        # partition-major tiling: qwt[p, g*NW+m] = qw2[g*128+p, m]
        # chunk-major: qwt row-block for chunk c is fully contiguous in DRAM
        qwt = np.ascontiguousarray(
            qw2.reshape(cfg.G // cfg.CH, cfg.CH, 128, cfg.NW)
            .transpose(0, 2, 1, 3)
            .reshape(cfg.G // cfg.CH, 128, cfg.CH * cfg.NW)
        ).reshape(128 * (cfg.G // cfg.CH), cfg.CH * cfg.NW)
        s_p = s_s[:, perm]
        spv = (s_p / e[None, :]).astype(np.float16)
        in_maps.append(
            {
                "qw": qwt,
                "qz": _pack_cols(znib_s),
                "sp": spv,
                "s2": (s_p - 1024.0 * spv.astype(np.float64)).astype(np.float32),
                "biasp": (-b_s[perm]).astype(np.float32),
                "xt": xt,
            }
        )
    return in_maps


def host_gather(cfg, results):
    perm = _perm(cfg)
    valid = perm < cfg.N_shard
    out = np.empty((cfg.T, cfg.N_shard * N_CORES), dtype=np.float32)
    for c in range(N_CORES):
        oT = results[c]["outT"]
        shard = np.empty((cfg.T, cfg.N_shard), dtype=np.float32)
        shard[:, perm[valid]] = oT[valid].T
        out[:, c * cfg.N_shard:(c + 1) * cfg.N_shard] = shard
    return out


# ---------------------------------------------------------------- device kernel


def build_kernel(nc, cfg, reps=1):
    f32, f16, i32 = mybir.dt.float32, mybir.dt.float16, mybir.dt.int32
    u8 = mybir.dt.uint8
    G, T, PW, NW, R, CH = cfg.G, cfg.T, cfg.PW, cfg.NW, cfg.R, cfg.CH
    NPAD = cfg.NPAD
    TPW = T * PW
    # mains strips of width SW (<=512) covering NPAD; the last strip
    # overlaps its predecessor so every PSUM column is written exactly
    SW = min(512, NPAD)
    SW2 = SW
    NSTR = -(-NPAD // SW)
    STRIPS = [(i * SW, 0) for i in range(NSTR - 1)]
    STRIPS.append((NPAD - SW, SW - (NPAD - (NSTR - 1) * SW)))
    PSTR = 32 * NSTR

    qw_d = nc.declare_dram_parameter("qw", [128 * (G // CH), CH * NW], i32, isOutput=False)
    qz_d = nc.declare_dram_parameter("qz", [G, NW], i32, isOutput=False)
    sp_d = nc.declare_dram_parameter("sp", [G, NPAD], f16, isOutput=False)
    s2_d = nc.declare_dram_parameter("s2", [G, NPAD], f32, isOutput=False)
    bias_d = nc.declare_dram_parameter("biasp", [NPAD], f32, isOutput=False)
    xt_d = nc.declare_dram_parameter("xt", [128, G * T], f32, isOutput=False)
    out_d = nc.declare_dram_parameter("outT", [NPAD, T], f32, isOutput=True)
    scr_d = nc.dram_tensor("scratch", [R, 128, 512], f16).ap()
    sco_d = nc.dram_tensor("scopy_dram", [PSTR, G * SW2], f16).ap()

    dmae = [nc.sync, nc.scalar]  # the two HWDGE rings

    def dq(i):
        return dmae[i % len(dmae)]

    with tile.TileContext(nc) as tc:
      for rep in range(reps):
       with ExitStack() as ctx:
        singles = ctx.enter_context(tc.tile_pool(name=f"singles{rep}", bufs=1))
        qwp = ctx.enter_context(tc.tile_pool(name=f"qwp{rep}", bufs=3))
        encp = ctx.enter_context(tc.tile_pool(name=f"encp{rep}", bufs=2))
        smallp = ctx.enter_context(tc.tile_pool(name=f"smallp{rep}", bufs=3))
        ps_main = ctx.enter_context(tc.tile_pool(name=f"ps_main{rep}", bufs=1, space="PSUM"))
        ps_sc = ctx.enter_context(tc.tile_pool(name=f"ps_sc{rep}", bufs=2, space="PSUM"))
        ps_c = ctx.enter_context(tc.tile_pool(name=f"ps_c{rep}", bufs=2, space="PSUM"))

        # ---------- phase 0: x prep ----------
        xf = singles.tile([128, G * T], f32)
        nc.sync.dma_start(out=xf[:], in_=xt_d[:])
        # tokens padded to 32 per group so matmuls write full PSUM strips
        xT = singles.tile([128, G * 32], f16)
        nc.vector.memset(xT[:], 0.0)
        nc.vector.tensor_copy(
            xT[:].rearrange("p (g t) -> p g t", g=G)[:, :, 0:T], xf[:]
        )
        ones16 = singles.tile([128, 1], f16)
        nc.vector.memset(ones16[:], 1.0)

        xsumP = ps_sc.tile([T, G], f32, tag="sc")
        for g in range(G):
            nc.tensor.matmul(
                xsumP[:, g:g + 1], xT[:, g * 32:g * 32 + T], ones16[:],
                start=True, stop=True,
            )
        xsum_s = singles.tile([T, G], f32)
        nc.scalar.copy(xsum_s[:], xsumP[:])
        ident = singles.tile([T, T], f32)
        make_identity(nc, ident[:])
        xsT_P = ps_sc.tile([G, T], f32, tag="sc")
        nc.tensor.transpose(xsT_P[:], xsum_s[:], ident[:])
        xsum_aug = singles.tile([G + 1, T], f32)
        nc.vector.memset(xsum_aug[:], 1.0)
        nc.scalar.copy(xsum_aug[:G, :], xsT_P[:])
        # offv[32j+t, g] = -1024 * xsum[t, g]
        offv = singles.tile([128, G], f32)
        nc.vector.memset(offv[:], 0.0)
        for j in range(4):
            nc.scalar.mul(offv[32 * j:32 * j + T, :], xsumP[:], -1024.0)

        # ---------- phase 1: scales / zeros prep ----------
        sp16 = singles.tile([G, NPAD], f16)
        nc.sync.dma_start(out=sp16[:], in_=sp_d[:])
        sp_s = singles.tile([G, NPAD], f32)
        nc.vector.tensor_copy(sp_s[:], sp16[:])
        s_s = singles.tile([G, NPAD], f32)
        nc.sync.dma_start(out=s_s[:], in_=s2_d[:])

        qz_t = singles.tile([G, NW], i32)
        nc.sync.dma_start(out=qz_t[:], in_=qz_d[:])
        zs = singles.tile([G, NW], i32)
        nc.vector.tensor_scalar(
            out=zs[:], in0=qz_t[:], scalar1=8, scalar2=None,
            op0=AluOpType.logical_shift_right,
        )
        zenc = singles.tile([G, NPAD], f16)
        zi = zenc[:].bitcast(i32)
        for j, (src, mask) in enumerate(
            [(qz_t, MASK_LO), (qz_t, MASK_HI), (zs, MASK_LO), (zs, MASK_HI)]
        ):
            nc.vector.tensor_scalar(
                out=zi[:, j * NW:(j + 1) * NW], in0=src[:],
                scalar1=mask, scalar2=EXP16,
                op0=AluOpType.bitwise_and, op1=AluOpType.bitwise_or,
            )
        zf = singles.tile([G, NPAD], f32)
        nc.vector.tensor_copy(zf[:], zenc[:])
        szb = singles.tile([G + 1, NPAD], f32)
        nc.vector.tensor_tensor(szb[:G, :], zf[:], sp_s[:], AluOpType.mult)
        nc.vector.tensor_tensor(szb[:G, :], szb[:G, :], s_s[:], AluOpType.add)
        nc.sync.dma_start(out=szb[G:G + 1, :], in_=bias_d[None, :])

        # ---------- phase 2: unpack + mains + evac ----------
        rhsbig = singles.tile([G, T * NPAD], f16)
        scopy = singles.tile([128, G * SW2], f16)
        nd = 0  # DMA ring round-robin counter
        for c0 in range(0, G, CH):
            wt = qwp.tile([128, CH * NW], i32, tag="wt")
            dq(nd).dma_start(
                out=wt[:], in_=qw_d[(c0 // CH) * 128:(c0 // CH + 1) * 128, :]
            )
            nd += 1
            ws = qwp.tile([128, CH * NW], i32, tag="ws")
            nc.vector.tensor_scalar(
                out=ws[:], in0=wt[:], scalar1=8, scalar2=None,
                op0=AluOpType.logical_shift_right,
            )
            enc = encp.tile([128, CH * NPAD], f16, tag="enc")
            ei = enc[:].bitcast(i32)
            for j, (src, mask) in enumerate(
                [(wt, MASK_LO), (wt, MASK_HI), (ws, MASK_LO), (ws, MASK_HI)]
            ):
                nc.vector.tensor_scalar(
                    out=ei[:].rearrange("p (g w) -> p g w", g=CH)[
                        :, :, j * NW:(j + 1) * NW
                    ],
                    in0=src[:],
                    scalar1=mask, scalar2=EXP16,
                    op0=AluOpType.bitwise_and, op1=AluOpType.bitwise_or,
                )
            for gg in range(CH):
                g = c0 + gg
                mainP = ps_main.tile(
                    [128, 512], f32, tag=f"m{g % 4}", name=f"mainP{rep}_{g % 4}"
                )
                for sidx, (c_lo, _) in enumerate(STRIPS):
                    nc.tensor.matmul(
                        mainP[32 * sidx:32 * (sidx + 1), 0:SW],
                        xT[:, g * 32:(g + 1) * 32],
                        enc[:, gg * NPAD + c_lo: gg * NPAD + c_lo + SW],
                        start=True, stop=True, tile_position=(0, 32 * sidx),
                    )
                # evac: subtract offset, fp16 (alternate DVE / ACT)
                if g % 2 == 0:
                    nc.vector.tensor_scalar(
                        out=scopy[0:PSTR, g * SW2:(g + 1) * SW2],
                        in0=mainP[0:PSTR, 0:SW],
                        scalar1=offv[0:PSTR, g:g + 1], scalar2=None,
                        op0=AluOpType.add,
                    )
                else:
                    nc.scalar.activation(
                        scopy[0:PSTR, g * SW2:(g + 1) * SW2], mainP[0:PSTR, 0:SW],
                        mybir.ActivationFunctionType.Identity,
                        bias=offv[0:PSTR, g:g + 1], scale=1.0,
                    )

            # chunk's evac window -> DRAM mirror (contiguous per partition)
            dq(nd).dma_start(
                out=sco_d[:, c0 * SW2:(c0 + CH) * SW2],
                in_=scopy[0:PSTR, c0 * SW2:(c0 + CH) * SW2],
            )
            nd += 1

        # remap gather: DRAM -> [g, (t, nflat)] tiles, one DMA per strip
        for sidx, (c_lo, f_lo) in enumerate(STRIPS):
            width = SW - f_lo
            dq(nd).dma_start(
                out=rhsbig[:, :].rearrange("g (t n) -> g t n", t=T)[
                    :, :, c_lo + f_lo: c_lo + f_lo + width
                ],
                in_=bass.AP(
                    tensor=sco_d.tensor,
                    offset=sco_d.offset + (32 * sidx) * (G * SW2) + f_lo,
                    ap=[[SW2, G], [G * SW2, T], [1, width]],
                ),
            )
            nd += 1

        # ---------- phase 3: scale matmul + correction + diag out ----------
        for u in range(R):
            scP = ps_sc.tile([128, 512], f32, tag="sc", name=f"scP{rep}_{u}")
            cP = ps_c.tile([128, T], f32, tag="c", name=f"cP{rep}_{u}")
            for v in range(4):
                w0 = 128 * u + 32 * v
                rhs_ap = rhsbig[:, :].rearrange("g (t n) -> g n t", t=T)[
                    :, w0:w0 + 32, :
                ]
                nc.tensor.matmul(
                    scP[32 * v:32 * (v + 1), :],
                    sp16[:, w0:w0 + 32],
                    rhs_ap,
                    start=True, stop=True, tile_position=(0, 32 * v),
                )
            nc.tensor.matmul(
                cP[:, :],
                szb[:, 128 * u:128 * (u + 1)],
                xsum_aug[:],
                start=True, stop=True,
            )
            scS = smallp.tile([128, 512], f16, tag="scS")
            if u % 2 == 0:
                nc.vector.tensor_copy(scS[:], scP[:])
            else:
                nc.scalar.copy(scS[:], scP[:])
            dq(nd).dma_start(out=scr_d[u], in_=scS[:])
            nd += 1
            diagbuf = smallp.tile([128, T], f16, tag="diagbuf")
            diag_src = bass.AP(
                tensor=scr_d.tensor,
                offset=scr_d.offset + u * 128 * 512,
                ap=[[32 * 512, 4], [512 + 16, 32], [1, T]],
            )
            dq(nd).dma_start(out=diagbuf[:], in_=diag_src)
            nd += 1
            oT = smallp.tile([128, T], f32, tag="oT")
            nc.vector.scalar_tensor_tensor(
                out=oT[:], in0=diagbuf[:], scalar=0.0, in1=cP[:],
                op0=AluOpType.bypass, op1=AluOpType.subtract,
            )
            dq(nd).dma_start(out=out_d[128 * u:128 * (u + 1), :], in_=oT[:])
            nd += 1
    return nc


# ---------------------------------------------------------------- entry

_CACHE = {}


def _get_nc(cfg):
    key = (cfg.K, cfg.NPAD, cfg.T)
    if key not in _CACHE:
        nc = bacc.Bacc(num_devices=N_CORES)
        build_kernel(nc, cfg)
        nc.compile()
        _CACHE[key] = nc
    return _CACHE[key]


def kernel(x, qweight, qzeros, scales, bias):
    cfg = FULL
    in_maps = host_prep(cfg, x, qweight, qzeros, scales, bias)
    nc = _get_nc(cfg)
    res = run_bass_kernel_spmd(nc, in_maps, core_ids=list(range(N_CORES)))
    return host_gather(cfg, res.results)
